# revision 37
# baseline (speedup 1.0000x reference)
"""AttentiveConv3d (sparse_attention) Trainium2 kernel — self-contained.

kernel(**inputs) takes the FULL inputs
    x     [2, 128, 16, 28, 28] f32
    q     [2, 1, 64] f32
    W_out [128, 128] f32
    b_out [128] f32
and returns the FULL output [2, 128, 16, 28, 28] f32.

Sharding: data-parallel over (batch, T-chunks): 8 cores, core i handles
batch i//4, output frames 4*(i%4) .. 4*(i%4)+3, with a 1-frame halo supplied
by host-side padding/slicing (no device collectives needed).

Math (equivalent to the reference; softmax computed without max-subtraction,
valid because |logits| < ~0.2 for this operator's scaling):
    z   = qmask^T @ xp        (per padded location; both heads)
    E   = exp(z);  F = E * xp
    num = Box3x3x3(F); d = Box3x3x3(E)    (separable box filters)
    y   = W_out @ (num / d) + b_out

v4: fp16 fields (f32 PSUM); host-padded input so exp(z)=1 / F=0 pads are
free; 36-row denominator pack (dy baked into 3 SBUF-SBUF DMAs, dx via 3
accumulating matmuls, one fused reciprocal); loads split across the SP and
ACT DMA queues to beat the ~650ns/DMA issue rate; PSUM pair tiles so the
merge/evacuate stages run as one op per frame; Pool (gpsimd) takes the
off-critical-path F multiplies and the f32r cast.
"""
from contextlib import ExitStack

import numpy as np

import concourse.bass as bass
import concourse.tile as tile
from concourse import bacc, mybir
from concourse import bass_utils

F32 = mybir.dt.float32
F32R = mybir.dt.float32r
F16 = mybir.dt.float16
AF = mybir.ActivationFunctionType

C = 128
TIN, TOUT = 6, 4
HP, WP = 30, 30
HO, WO = 28, 28
NF = HP * WP        # 900
NOF = HO * WO       # 784


def _build_nc(num_devices=8, reps=1, n_warm=4,
              f_pool=(0, 1), w_pool=(), h_pool=()):
    nc = bacc.Bacc("TRN2", target_bir_lowering=False, debug=False,
                   num_devices=num_devices)
    d_xp = nc.dram_tensor("xp", [C, TIN, NF], F16, kind="ExternalInput").ap()
    # cst: qm | idm | wt | sel36 (rows 0:36 of col 384:392) | bias f16 (col 392)
    d_cst = nc.dram_tensor("cst", [C, 393], F16, kind="ExternalInput").ap()
    d_selr = nc.dram_tensor("selr", [8, TOUT * C], F32R, kind="ExternalInput").ap()
    d_ys = [nc.dram_tensor(f"y{t}", [C, NOF], F16, kind="ExternalOutput").ap()
            for t in range(TOUT)]

    with tile.TileContext(nc) as tc:
        with ExitStack() as ctx:
            consts = ctx.enter_context(tc.tile_pool(name="consts", bufs=1))
            sb_x = ctx.enter_context(tc.tile_pool(name="sb_x", bufs=1))
            sb_e = ctx.enter_context(tc.tile_pool(name="sb_e", bufs=1))
            sb_f = ctx.enter_context(tc.tile_pool(name="sb_f", bufs=1))
            sb_s = ctx.enter_context(tc.tile_pool(name="sb_s", bufs=3))
            sb_tmp = ctx.enter_context(tc.tile_pool(name="sb_tmp", bufs=3))
            sb_m = ctx.enter_context(tc.tile_pool(name="sb_m", bufs=2))
            sb_y = ctx.enter_context(tc.tile_pool(name="sb_y", bufs=4))
            sb_ep = ctx.enter_context(tc.tile_pool(name="sb_ep", bufs=1))
            ps_big = ctx.enter_context(tc.tile_pool(name="ps_big", bufs=3, space="PSUM"))
            ps_small = ctx.enter_context(tc.tile_pool(name="ps_small", bufs=2, space="PSUM"))

            cst_t = consts.tile([C, 393], F16)
            selr_t = consts.tile([8, TOUT * C], F32R)
            bias_t = consts.tile([C, 1], F32)

            # Warm-up: ramp the PE p-state on a memset dummy (no DMA dep).
            wrm_sb = consts.tile([C, 512], F16)
            nc.gpsimd.memset(wrm_sb[:], 0.0)
            for i in range(n_warm):
                wrm = ps_small.tile([C, 512], F32, tag="small", name=f"warm{i}")
                nc.tensor.matmul(wrm[:], wrm_sb[:, 0:128], wrm_sb[:],
                                 start=True, stop=True)

            qm = cst_t[:, 0:128]
            idm = cst_t[:, 128:256]
            wt = cst_t[:, 256:384]
            sel36 = cst_t[0:36, 384:392]
            bias = bias_t[:]
            selr = selr_t[:].rearrange("p (t c) -> p t c", t=TOUT)

            for _ in range(reps):
                _body(tc, nc, d_xp, d_ys, d_selr, d_cst,
                      qm, idm, wt, bias, selr, selr_t, sel36, cst_t,
                      sb_x, sb_e, sb_f, sb_s, sb_tmp, sb_m, sb_y, sb_ep,
                      ps_big, ps_small, f_pool, w_pool, h_pool)
    nc.compile()
    return nc


def _body(tc, nc, d_xp, d_ys, d_selr, d_cst,
          qm, idm, wt, bias, selr, selr_t, sel36, cst_t,
          sb_x, sb_e, sb_f, sb_s, sb_tmp, sb_m, sb_y, sb_ep,
          ps_big, ps_small, f_pool, w_pool, h_pool):
    e128 = sb_e.tile([C, TIN * NF], F16, tag="e128")
    ep36 = sb_ep.tile([36, NF], F16, tag="ep36")
    f_tiles = []
    num_tiles = {}
    x_tiles = {}

    def _load(f):
        xtt = sb_x.tile([C, NF], F16, tag=f"x{f}", name=f"xt{f}")
        nc.sync.dma_start(out=xtt[:], in_=d_xp[:, f])
        x_tiles[f] = xtt[:]

    def _frame(f):
        """z matmul, E = exp(z), F = E*xp."""
        xt = x_tiles[f]
        zp = ps_big.tile([C, 1024], F32, tag="big", name=f"zp{f}")
        nc.tensor.matmul(zp[:, 0:512], qm, xt[:, 0:512], start=True, stop=True)
        nc.tensor.matmul(zp[:, 512:900], qm, xt[:, 512:900], start=True, stop=True)

        # exp over the full padded frame: z=0 at pads -> E=1 there for free
        ef = e128[:, f * NF:(f + 1) * NF]
        nc.scalar.activation(ef, zp[:, 0:900], AF.Exp)

        # F = E * xp over the full frame: xp pads are 0 -> F=0 pads for free
        ft = sb_f.tile([C, NF], F16, tag=f"f{f}", name=f"ft{f}")
        feng = nc.gpsimd if f in f_pool else nc.vector
        feng.tensor_mul(ft[:], ef, xt)
        f_tiles.append(ft)

    def _numpart(t):
        """T-pass (PE) -> S evac (ACT) -> W -> H (DVE/Pool)."""
        ftp = ps_big.tile([C, 1024], F32, tag="big", name=f"ftp{t}")
        for half in range(2):
            lo = half * 512
            n = 512 if half == 0 else 388
            for dt in range(3):
                nc.tensor.matmul(ftp[:, lo:lo + n], idm,
                                 f_tiles[t + dt][:, lo:lo + n],
                                 start=(dt == 0), stop=(dt == 2))
        st = sb_s.tile([C, NF], F16, tag="s", name=f"st{t}")
        nc.scalar.activation(st[:], ftp[:, 0:900], AF.Copy)

        sv = st[:].rearrange("p (y x) -> p y x", y=HP)
        weng = nc.gpsimd if t in w_pool else nc.vector
        w1 = sb_tmp.tile([C, HP, WO], F16, tag="w1", name=f"w1_{t}")
        weng.tensor_add(w1[:], sv[:, :, 0:28], sv[:, :, 1:29])
        w2 = sb_tmp.tile([C, HP, WO], F16, tag="w2", name=f"w2_{t}")
        weng.tensor_add(w2[:], w1[:], sv[:, :, 2:30])

        heng = nc.gpsimd if t in h_pool else nc.vector
        h1 = sb_tmp.tile([C, HO, WO], F16, tag="h1", name=f"h1_{t}")
        heng.tensor_add(h1[:], w2[:, 0:28, :], w2[:, 1:29, :])
        numt = sb_tmp.tile([C, HO, WO], F16, tag="numt", name=f"numt{t}")
        heng.tensor_add(numt[:], h1[:], w2[:, 2:30, :])
        num_tiles[t] = numt

    _load(0)
    nc.sync.dma_start(out=cst_t[:], in_=d_cst[:])
    nc.gpsimd.tensor_scalar_add(bias[0:C, 0:1], cst_t[:, 392:393], 0.0)
    for f in range(1, TIN):
        _load(f)
    nc.sync.dma_start(out=selr_t[:], in_=d_selr[:])
    for f in range(TIN):
        _frame(f)

    # ---- denominator: 3 dy-packs (rows 12dy+6h+t), 3 dx matmuls, one
    # fused reciprocal ----
    ebase = e128[:]
    epbase = ep36[:]
    estride = ebase.ap[0][0]
    epstride = epbase.ap[0][0]
    for dy in range(3):
        src = bass.AP(tensor=ebase.tensor, offset=ebase.offset + 30 * dy,
                      ap=[[estride, 2], [NF, TIN], [1, 840]])
        dst = bass.AP(tensor=epbase.tensor,
                      offset=epbase.offset + 12 * dy * epstride,
                      ap=[[epstride, 12], [1, 840]])
        nc.sync.dma_start(out=dst, in_=src)

    d8p = ps_big.tile([8, 1024], F32, tag="big", name="d8p")
    for ch in range(2):
        ylo = 14 * ch
        for dx in range(3):
            rhs = bass.AP(tensor=epbase.tensor,
                          offset=epbase.offset + 30 * ylo + dx,
                          ap=[[epstride, 36], [30, 14], [1, WO]])
            nc.tensor.matmul(d8p[:, 512 * ch:512 * ch + 392], sel36, rhs,
                             start=(dx == 0), stop=(dx == 2))
    r8f = sb_ep.tile([8, NOF], F32, tag="r8f")
    d8v = d8p[:].rearrange("p (b k) -> p b k", b=2)[:, :, 0:392]
    r8v = r8f[:].rearrange("p (b k) -> p b k", b=2)
    nc.vector.reciprocal_approx_fast(r8v, d8v)
    r8t = sb_ep.tile([8, NOF], F32R, tag="r8")
    nc.vector.tensor_scalar_add(r8t[:], r8f[:], 0.0)
    r8 = r8t[:]

    def _tail(t):
        nv = num_tiles[t][:].rearrange("p y x -> p (y x)")
        mt = sb_m.tile([C, NOF], F16, tag="m", name=f"mt{t}")
        yt = sb_y.tile([C, NOF], F16, tag="y", name=f"yt{t}")

        rp = ps_big.tile([C, 1024], F32, tag="big", name=f"rp{t}")
        for ch in range(2):
            nc.tensor.matmul(rp[:, 512 * ch:512 * ch + 392], selr[:, t, :],
                             r8[:, ch * 392:ch * 392 + 392],
                             start=True, stop=True)
        rpv = rp[:].rearrange("p (b k) -> p b k", b=2)[:, :, 0:392]
        nc.vector.tensor_mul(mt[:].rearrange("p (b k) -> p b k", b=2), nv, rpv)

        yp = ps_big.tile([C, 1024], F32, tag="big", name=f"yp{t}")
        for ch in range(2):
            nc.tensor.matmul(yp[:, 512 * ch:512 * ch + 392], wt,
                             mt[:, ch * 392:ch * 392 + 392],
                             start=True, stop=True)
        ypv = yp[:].rearrange("p (b k) -> p b k", b=2)[:, :, 0:392]
        nc.scalar.activation(yt[:].rearrange("p (b k) -> p b k", b=2), ypv,
                             AF.Identity, bias=bias, scale=1.0)
        nc.gpsimd.dma_start(out=d_ys[t][:], in_=yt[:])

    # ---- numparts and tails interleaved: mt_t follows H_t on DVE ----
    for t in range(TOUT):
        _numpart(t)
        _tail(t)


# ---------------------------------------------------------------------------
# Host side
# ---------------------------------------------------------------------------

def _host_prep(x, q, W_out, b_out):
    B, C_, T, H, W = x.shape
    heads, hs = 2, 64
    xpad = np.zeros((B, C_, T + 2, HP, WP), np.float16)
    xpad[:, :, 1:T + 1, 1:H + 1, 1:W + 1] = np.asarray(x, np.float32)

    cidx = np.arange(C_)
    qfull = (np.asarray(q, np.float32)[cidx % heads, 0, cidx // heads] / hs)
    qm = np.zeros((C_, C_), np.float32)
    for m in range(C_):
        qm[:, m] = np.where(cidx % heads == m % heads, qfull, 0.0)
    cst = np.zeros((C_, 393), np.float16)
    cst[:, 0:128] = qm
    cst[:, 128:256] = np.eye(C_)
    cst[:, 256:384] = np.asarray(W_out, np.float32).T
    cst[:, 392] = np.asarray(b_out, np.float32)

    # sel36[12dy + 6h + t, 4h + tp] = 1 iff 0 <= t - tp <= 2
    for dy in range(3):
        for h in range(2):
            for t in range(TIN):
                for tp in range(TOUT):
                    if 0 <= t - tp <= 2:
                        cst[12 * dy + 6 * h + t, 384 + 4 * h + tp] = 1.0
    # selr[4h + tp, tp, c] = 1 iff c % heads == h   (r-broadcast select)
    selr = np.zeros((8, TOUT, C_), np.float32)
    for tp in range(TOUT):
        selr[4 * (cidx % heads) + tp, tp, cidx] = 1.0

    shared = {"cst": cst, "selr": selr.reshape(8, TOUT * C_)}
    in_maps = []
    for core in range(8):
        b, t0 = core // 4, (core % 4) * 4
        xp = np.ascontiguousarray(
            xpad[b, :, t0:t0 + TIN].reshape(C_, TIN, NF))
        in_maps.append({"xp": xp, **shared})
    return in_maps


_NC_CACHE = {}


def _get_nc(reps=1):
    if reps not in _NC_CACHE:
        _NC_CACHE[reps] = _build_nc(reps=reps)
    return _NC_CACHE[reps]


def kernel(x, q, W_out, b_out):
    x = np.asarray(x, np.float32)
    in_maps = _host_prep(x, q, W_out, b_out)
    nc = _get_nc()
    res = bass_utils.run_bass_kernel_spmd(nc, in_maps, list(range(8)))
    y = np.zeros((2, 128, 16, 28, 28), np.float32)
    for core in range(8):
        b, t0 = core // 4, (core % 4) * 4
        for t in range(TOUT):
            y[b, :, t0 + t] = np.asarray(
                res.results[core][f"y{t}"], np.float32).reshape(C, HO, WO)
    return y


# revision 40
# speedup vs baseline: 1.6169x; 1.6169x over previous
"""AttentiveConv3d (sparse_attention) Trainium2 kernel — self-contained.

kernel(**inputs) takes the FULL inputs
    x     [2, 128, 16, 28, 28] f32
    q     [2, 1, 64] f32
    W_out [128, 128] f32
    b_out [128] f32
and returns the FULL output [2, 128, 16, 28, 28] f32.

Sharding: data-parallel over (batch, T-chunks): 8 cores, core i handles
batch i//4, output frames 4*(i%4) .. 4*(i%4)+3, with a 1-frame halo supplied
by host-side padding/slicing (no device collectives needed).

Math (equivalent to the reference; softmax computed without max-subtraction,
valid because |logits| < ~0.2 for this operator's scaling):
    z   = qmask^T @ xp        (per padded location; both heads)
    E   = exp(z);  F = E * xp
    num = Box3x3x3(F); d = Box3x3x3(E)    (separable box filters)
    y   = W_out @ (num / d) + b_out

v4: fp16 fields (f32 PSUM); host-padded input so exp(z)=1 / F=0 pads are
free; 36-row denominator pack (dy baked into 3 SBUF-SBUF DMAs, dx via 3
accumulating matmuls, one fused reciprocal); loads split across the SP and
ACT DMA queues to beat the ~650ns/DMA issue rate; PSUM pair tiles so the
merge/evacuate stages run as one op per frame; Pool (gpsimd) takes the
off-critical-path F multiplies and the f32r cast.
"""
from contextlib import ExitStack

import numpy as np

import concourse.bass as bass
import concourse.tile as tile
from concourse import bacc, mybir
from concourse import bass_utils

F32 = mybir.dt.float32
F32R = mybir.dt.float32r
F16 = mybir.dt.float16
AF = mybir.ActivationFunctionType

C = 128
TIN, TOUT = 6, 4
HP, WP = 30, 30
HO, WO = 28, 28
NF = HP * WP        # 900
NOF = HO * WO       # 784


def _build_nc(num_devices=8, reps=1, n_warm=4,
              f_pool=(0, 1), w_pool=(), h_pool=()):
    nc = bacc.Bacc("TRN2", target_bir_lowering=False, debug=False,
                   num_devices=num_devices)
    d_xp = nc.dram_tensor("xp", [C, TIN, NF], F16, kind="ExternalInput").ap()
    # cst: qm | idm | wt | sel36 (rows 0:36 of col 384:392) | bias f16 (col 392)
    d_cst = nc.dram_tensor("cst", [C, 393], F16, kind="ExternalInput").ap()
    d_selr = nc.dram_tensor("selr", [8, TOUT * C], F32R, kind="ExternalInput").ap()
    d_ys = [nc.dram_tensor(f"y{t}", [C, NOF], F16, kind="ExternalOutput").ap()
            for t in range(TOUT)]

    with tile.TileContext(nc) as tc:
        with ExitStack() as ctx:
            consts = ctx.enter_context(tc.tile_pool(name="consts", bufs=1))
            sb_x = ctx.enter_context(tc.tile_pool(name="sb_x", bufs=1))
            sb_e = ctx.enter_context(tc.tile_pool(name="sb_e", bufs=1))
            sb_f = ctx.enter_context(tc.tile_pool(name="sb_f", bufs=1))
            sb_s = ctx.enter_context(tc.tile_pool(name="sb_s", bufs=3))
            sb_tmp = ctx.enter_context(tc.tile_pool(name="sb_tmp", bufs=3))
            sb_m = ctx.enter_context(tc.tile_pool(name="sb_m", bufs=2))
            sb_y = ctx.enter_context(tc.tile_pool(name="sb_y", bufs=4))
            sb_ep = ctx.enter_context(tc.tile_pool(name="sb_ep", bufs=1))
            ps_a = ctx.enter_context(tc.tile_pool(name="ps_a", bufs=2, space="PSUM"))
            ps_b = ctx.enter_context(tc.tile_pool(name="ps_b", bufs=2, space="PSUM"))

            cst_t = consts.tile([C, 393], F16)
            selr_t = consts.tile([8, TOUT * C], F32R)
            bias_t = consts.tile([C, 1], F32)

            # Warm-up: ramp the PE p-state on a memset dummy (no DMA dep).
            wrm_sb = consts.tile([C, 512], F16)
            nc.gpsimd.memset(wrm_sb[:], 0.0)
            for i in range(n_warm):
                wrm = ps_a.tile([C, 1024], F32, tag="a", name=f"warm{i}")
                nc.tensor.matmul(wrm[:, 0:512], wrm_sb[:, 0:128], wrm_sb[:],
                                 start=True, stop=True)

            qm = cst_t[:, 0:128]
            idm = cst_t[:, 128:256]
            wt = cst_t[:, 256:384]
            sel36 = cst_t[0:36, 384:392]
            bias = bias_t[:]
            selr = selr_t[:].rearrange("p (t c) -> p t c", t=TOUT)

            for _ in range(reps):
                _body(tc, nc, d_xp, d_ys, d_selr, d_cst,
                      qm, idm, wt, bias, selr, selr_t, sel36, cst_t,
                      sb_x, sb_e, sb_f, sb_s, sb_tmp, sb_m, sb_y, sb_ep,
                      ps_a, ps_b, f_pool, w_pool, h_pool)
    nc.compile()
    return nc


def _body(tc, nc, d_xp, d_ys, d_selr, d_cst,
          qm, idm, wt, bias, selr, selr_t, sel36, cst_t,
          sb_x, sb_e, sb_f, sb_s, sb_tmp, sb_m, sb_y, sb_ep,
          ps_a, ps_b, f_pool, w_pool, h_pool):
    e128 = sb_e.tile([C, TIN * NF], F16, tag="e128")
    ep36 = sb_ep.tile([36, NF], F16, tag="ep36")
    f_tiles = []
    num_tiles = {}
    x_tiles = {}

    def _load(f):
        xtt = sb_x.tile([C, NF], F16, tag=f"x{f}", name=f"xt{f}")
        nc.sync.dma_start(out=xtt[:], in_=d_xp[:, f])
        x_tiles[f] = xtt[:]

    def _frame(f):
        """z matmul, E = exp(z), F = E*xp."""
        xt = x_tiles[f]
        zp = ps_a.tile([C, 1024], F32, tag="a", name=f"zp{f}")
        nc.tensor.matmul(zp[:, 0:512], qm, xt[:, 0:512], start=True, stop=True)
        nc.tensor.matmul(zp[:, 512:900], qm, xt[:, 512:900], start=True, stop=True)

        # exp over the full padded frame: z=0 at pads -> E=1 there for free
        ef = e128[:, f * NF:(f + 1) * NF]
        nc.scalar.activation(ef, zp[:, 0:900], AF.Exp)

        # F = E * xp over the full frame: xp pads are 0 -> F=0 pads for free
        ft = sb_f.tile([C, NF], F16, tag=f"f{f}", name=f"ft{f}")
        feng = nc.gpsimd if f in f_pool else nc.vector
        feng.tensor_mul(ft[:], ef, xt)
        f_tiles.append(ft)

    def _numpart(t):
        """T-pass (PE) -> S evac (ACT) -> W -> H (DVE/Pool)."""
        ftp = ps_b.tile([C, 1024], F32, tag="b", name=f"ftp{t}")
        for half in range(2):
            lo = half * 512
            n = 512 if half == 0 else 388
            for dt in range(3):
                nc.tensor.matmul(ftp[:, lo:lo + n], idm,
                                 f_tiles[t + dt][:, lo:lo + n],
                                 start=(dt == 0), stop=(dt == 2))
        st = sb_s.tile([C, NF], F16, tag="s", name=f"st{t}")
        nc.scalar.activation(st[:], ftp[:, 0:900], AF.Copy)

        sv = st[:].rearrange("p (y x) -> p y x", y=HP)
        weng = nc.gpsimd if t in w_pool else nc.vector
        w1 = sb_tmp.tile([C, HP, WO], F16, tag="w1", name=f"w1_{t}")
        weng.tensor_add(w1[:], sv[:, :, 0:28], sv[:, :, 1:29])
        w2 = sb_tmp.tile([C, HP, WO], F16, tag="w2", name=f"w2_{t}")
        weng.tensor_add(w2[:], w1[:], sv[:, :, 2:30])

        heng = nc.gpsimd if t in h_pool else nc.vector
        h1 = sb_tmp.tile([C, HO, WO], F16, tag="h1", name=f"h1_{t}")
        heng.tensor_add(h1[:], w2[:, 0:28, :], w2[:, 1:29, :])
        numt = sb_tmp.tile([C, HO, WO], F16, tag="numt", name=f"numt{t}")
        heng.tensor_add(numt[:], h1[:], w2[:, 2:30, :])
        num_tiles[t] = numt

    _load(0)
    nc.sync.dma_start(out=cst_t[:], in_=d_cst[:])
    nc.gpsimd.tensor_scalar_add(bias[0:C, 0:1], cst_t[:, 392:393], 0.0)
    for f in range(1, TIN):
        _load(f)
    nc.sync.dma_start(out=selr_t[:], in_=d_selr[:])

    # creation order fixes the PSUM rotation; priority gaps let the tail
    # ops (issued later) slot in right behind their frame's numpart
    prio_gap = {}
    for f in range(2):
        _frame(f)
    for t in range(TOUT):
        _frame(t + 2)
        _numpart(t)
        if t == 0:
            prio_den = tc.cur_priority
            tc.cur_priority += 24
        prio_gap[t] = tc.cur_priority
        tc.cur_priority += 24

    # ---- denominator: 3 dy-packs (rows 12dy+6h+t), 3 dx matmuls, one
    # fused reciprocal ----  (priority: right after tail0's slot)
    saved_p = tc.cur_priority
    tc.cur_priority = prio_den
    ebase = e128[:]
    epbase = ep36[:]
    estride = ebase.ap[0][0]
    epstride = epbase.ap[0][0]
    for dy in range(3):
        src = bass.AP(tensor=ebase.tensor, offset=ebase.offset + 30 * dy,
                      ap=[[estride, 2], [NF, TIN], [1, 840]])
        dst = bass.AP(tensor=epbase.tensor,
                      offset=epbase.offset + 12 * dy * epstride,
                      ap=[[epstride, 12], [1, 840]])
        nc.sync.dma_start(out=dst, in_=src)

    d8p = ps_a.tile([8, 1024], F32, tag="a", name="d8p")
    for ch in range(2):
        ylo = 14 * ch
        for dx in range(3):
            rhs = bass.AP(tensor=epbase.tensor,
                          offset=epbase.offset + 30 * ylo + dx,
                          ap=[[epstride, 36], [30, 14], [1, WO]])
            nc.tensor.matmul(d8p[:, 512 * ch:512 * ch + 392], sel36, rhs,
                             start=(dx == 0), stop=(dx == 2))
    r8f = sb_ep.tile([8, NOF], F32, tag="r8f")
    d8v = d8p[:].rearrange("p (b k) -> p b k", b=2)[:, :, 0:392]
    r8v = r8f[:].rearrange("p (b k) -> p b k", b=2)
    nc.vector.reciprocal_approx_fast(r8v, d8v)
    r8t = sb_ep.tile([8, NOF], F32R, tag="r8")
    nc.vector.tensor_scalar_add(r8t[:], r8f[:], 0.0)
    r8 = r8t[:]
    tc.cur_priority = saved_p

    def _tail(t):
        nv = num_tiles[t][:].rearrange("p y x -> p (y x)")
        mt = sb_m.tile([C, NOF], F16, tag="m", name=f"mt{t}")
        yt = sb_y.tile([C, NOF], F16, tag="y", name=f"yt{t}")

        rp = ps_a.tile([C, 1024], F32, tag="a", name=f"rp{t}")
        for ch in range(2):
            nc.tensor.matmul(rp[:, 512 * ch:512 * ch + 392], selr[:, t, :],
                             r8[:, ch * 392:ch * 392 + 392],
                             start=True, stop=True)
        rpv = rp[:].rearrange("p (b k) -> p b k", b=2)[:, :, 0:392]
        nc.vector.tensor_mul(mt[:].rearrange("p (b k) -> p b k", b=2), nv, rpv)

        yp = ps_a.tile([C, 1024], F32, tag="a", name=f"yp{t}")
        for ch in range(2):
            nc.tensor.matmul(yp[:, 512 * ch:512 * ch + 392], wt,
                             mt[:, ch * 392:ch * 392 + 392],
                             start=True, stop=True)
        ypv = yp[:].rearrange("p (b k) -> p b k", b=2)[:, :, 0:392]
        nc.scalar.activation(yt[:].rearrange("p (b k) -> p b k", b=2), ypv,
                             AF.Identity, bias=bias, scale=1.0)
        nc.gpsimd.dma_start(out=d_ys[t][:], in_=yt[:])

    # tails: creation order after everything (PSUM rotation), priority
    # slotted right behind each frame's numpart
    for t in range(TOUT):
        saved_p = tc.cur_priority
        tc.cur_priority = prio_gap[t]
        _tail(t)
        tc.cur_priority = saved_p


# ---------------------------------------------------------------------------
# Host side
# ---------------------------------------------------------------------------

def _host_prep(x, q, W_out, b_out):
    B, C_, T, H, W = x.shape
    heads, hs = 2, 64
    xpad = np.zeros((B, C_, T + 2, HP, WP), np.float16)
    xpad[:, :, 1:T + 1, 1:H + 1, 1:W + 1] = np.asarray(x, np.float32)

    cidx = np.arange(C_)
    qfull = (np.asarray(q, np.float32)[cidx % heads, 0, cidx // heads] / hs)
    qm = np.zeros((C_, C_), np.float32)
    for m in range(C_):
        qm[:, m] = np.where(cidx % heads == m % heads, qfull, 0.0)
    cst = np.zeros((C_, 393), np.float16)
    cst[:, 0:128] = qm
    cst[:, 128:256] = np.eye(C_)
    cst[:, 256:384] = np.asarray(W_out, np.float32).T
    cst[:, 392] = np.asarray(b_out, np.float32)

    # sel36[12dy + 6h + t, 4h + tp] = 1 iff 0 <= t - tp <= 2
    for dy in range(3):
        for h in range(2):
            for t in range(TIN):
                for tp in range(TOUT):
                    if 0 <= t - tp <= 2:
                        cst[12 * dy + 6 * h + t, 384 + 4 * h + tp] = 1.0
    # selr[4h + tp, tp, c] = 1 iff c % heads == h   (r-broadcast select)
    selr = np.zeros((8, TOUT, C_), np.float32)
    for tp in range(TOUT):
        selr[4 * (cidx % heads) + tp, tp, cidx] = 1.0

    shared = {"cst": cst, "selr": selr.reshape(8, TOUT * C_)}
    in_maps = []
    for core in range(8):
        b, t0 = core // 4, (core % 4) * 4
        xp = np.ascontiguousarray(
            xpad[b, :, t0:t0 + TIN].reshape(C_, TIN, NF))
        in_maps.append({"xp": xp, **shared})
    return in_maps


_NC_CACHE = {}


def _get_nc(reps=1):
    if reps not in _NC_CACHE:
        _NC_CACHE[reps] = _build_nc(reps=reps)
    return _NC_CACHE[reps]


def kernel(x, q, W_out, b_out):
    x = np.asarray(x, np.float32)
    in_maps = _host_prep(x, q, W_out, b_out)
    nc = _get_nc()
    res = bass_utils.run_bass_kernel_spmd(nc, in_maps, list(range(8)))
    y = np.zeros((2, 128, 16, 28, 28), np.float32)
    for core in range(8):
        b, t0 = core // 4, (core % 4) * 4
        for t in range(TOUT):
            y[b, :, t0 + t] = np.asarray(
                res.results[core][f"y{t}"], np.float32).reshape(C, HO, WO)
    return y


# revision 41
# speedup vs baseline: 1.8679x; 1.1552x over previous
"""AttentiveConv3d (sparse_attention) Trainium2 kernel — self-contained.

kernel(**inputs) takes the FULL inputs
    x     [2, 128, 16, 28, 28] f32
    q     [2, 1, 64] f32
    W_out [128, 128] f32
    b_out [128] f32
and returns the FULL output [2, 128, 16, 28, 28] f32.

Sharding: data-parallel over (batch, T-chunks): 8 cores, core i handles
batch i//4, output frames 4*(i%4) .. 4*(i%4)+3, with a 1-frame halo supplied
by host-side padding/slicing (no device collectives needed).

Math (equivalent to the reference; softmax computed without max-subtraction,
valid because |logits| < ~0.2 for this operator's scaling):
    z   = qmask^T @ xp        (per padded location; both heads)
    E   = exp(z);  F = E * xp
    num = Box3x3x3(F); d = Box3x3x3(E)    (separable box filters)
    y   = W_out @ (num / d) + b_out

v4: fp16 fields (f32 PSUM); host-padded input so exp(z)=1 / F=0 pads are
free; 36-row denominator pack (dy baked into 3 SBUF-SBUF DMAs, dx via 3
accumulating matmuls, one fused reciprocal); loads split across the SP and
ACT DMA queues to beat the ~650ns/DMA issue rate; PSUM pair tiles so the
merge/evacuate stages run as one op per frame; Pool (gpsimd) takes the
off-critical-path F multiplies and the f32r cast.
"""
from contextlib import ExitStack

import numpy as np

import concourse.bass as bass
import concourse.tile as tile
from concourse import bacc, mybir
from concourse import bass_utils

F32 = mybir.dt.float32
F32R = mybir.dt.float32r
F16 = mybir.dt.float16
AF = mybir.ActivationFunctionType

C = 128
TIN, TOUT = 6, 4
HP, WP = 30, 30
HO, WO = 28, 28
NF = HP * WP        # 900
NOF = HO * WO       # 784


def _build_nc(num_devices=8, reps=1, n_warm=4,
              f_pool=(0, 1), w_pool=(), h_pool=()):
    nc = bacc.Bacc("TRN2", target_bir_lowering=False, debug=False,
                   num_devices=num_devices)
    d_xp = nc.dram_tensor("xp", [C, TIN, NF], F16, kind="ExternalInput").ap()
    # cst: qm | idm | wt | sel36 (rows 0:36 of col 384:392) | bias f16 (col 392)
    d_cst = nc.dram_tensor("cst", [C, 393], F16, kind="ExternalInput").ap()
    d_selr = nc.dram_tensor("selr", [8, TOUT * C], F32R, kind="ExternalInput").ap()
    d_ys = [nc.dram_tensor(f"y{t}", [C, NOF], F16, kind="ExternalOutput").ap()
            for t in range(TOUT)]

    with tile.TileContext(nc) as tc:
        with ExitStack() as ctx:
            consts = ctx.enter_context(tc.tile_pool(name="consts", bufs=1))
            sb_x = ctx.enter_context(tc.tile_pool(name="sb_x", bufs=1))
            sb_e = ctx.enter_context(tc.tile_pool(name="sb_e", bufs=1))
            sb_f = ctx.enter_context(tc.tile_pool(name="sb_f", bufs=1))
            sb_s = ctx.enter_context(tc.tile_pool(name="sb_s", bufs=3))
            sb_tmp = ctx.enter_context(tc.tile_pool(name="sb_tmp", bufs=3))
            sb_m = ctx.enter_context(tc.tile_pool(name="sb_m", bufs=2))
            sb_y = ctx.enter_context(tc.tile_pool(name="sb_y", bufs=4))
            sb_ep = ctx.enter_context(tc.tile_pool(name="sb_ep", bufs=1))
            ps_a = ctx.enter_context(tc.tile_pool(name="ps_a", bufs=2, space="PSUM"))
            ps_b = ctx.enter_context(tc.tile_pool(name="ps_b", bufs=2, space="PSUM"))

            cst_t = consts.tile([C, 393], F16)
            selr_t = consts.tile([8, TOUT * C], F32R)
            bias_t = consts.tile([C, 1], F32)

            # Warm-up: ramp the PE p-state on a memset dummy (no DMA dep).
            wrm_sb = consts.tile([C, 512], F16)
            nc.gpsimd.memset(wrm_sb[:], 0.0)
            for i in range(n_warm):
                wrm = ps_a.tile([C, 1024], F32, tag="a", name=f"warm{i}")
                nc.tensor.matmul(wrm[:, 0:512], wrm_sb[:, 0:128], wrm_sb[:],
                                 start=True, stop=True)

            qm = cst_t[:, 0:128]
            idm = cst_t[:, 128:256]
            wt = cst_t[:, 256:384]
            sel36 = cst_t[0:36, 384:392]
            bias = bias_t[:]
            selr = selr_t[:].rearrange("p (t c) -> p t c", t=TOUT)

            for _ in range(reps):
                _body(tc, nc, d_xp, d_ys, d_selr, d_cst,
                      qm, idm, wt, bias, selr, selr_t, sel36, cst_t,
                      sb_x, sb_e, sb_f, sb_s, sb_tmp, sb_m, sb_y, sb_ep,
                      ps_a, ps_b, f_pool, w_pool, h_pool)
    nc.compile()
    return nc


def _body(tc, nc, d_xp, d_ys, d_selr, d_cst,
          qm, idm, wt, bias, selr, selr_t, sel36, cst_t,
          sb_x, sb_e, sb_f, sb_s, sb_tmp, sb_m, sb_y, sb_ep,
          ps_a, ps_b, f_pool, w_pool, h_pool):
    e128 = sb_e.tile([C, TIN * NF], F16, tag="e128")
    ep36 = sb_ep.tile([36, NF], F16, tag="ep36")
    f_tiles = []
    num_tiles = {}
    x_tiles = {}

    def _load(f):
        xtt = sb_x.tile([C, NF], F16, tag=f"x{f}", name=f"xt{f}")
        nc.sync.dma_start(out=xtt[:], in_=d_xp[:, f])
        x_tiles[f] = xtt[:]

    def _frame(f):
        """z matmul, E = exp(z), F = E*xp."""
        xt = x_tiles[f]
        zp = ps_a.tile([C, 1024], F32, tag="a", name=f"zp{f}")
        nc.tensor.matmul(zp[:, 0:512], qm, xt[:, 0:512], start=True, stop=True)
        nc.tensor.matmul(zp[:, 512:900], qm, xt[:, 512:900], start=True, stop=True)

        # exp over the full padded frame: z=0 at pads -> E=1 there for free
        ef = e128[:, f * NF:(f + 1) * NF]
        nc.scalar.activation(ef, zp[:, 0:900], AF.Exp)

        # F = E * xp over the full frame: xp pads are 0 -> F=0 pads for free
        ft = sb_f.tile([C, NF], F16, tag=f"f{f}", name=f"ft{f}")
        feng = nc.gpsimd if f in f_pool else nc.vector
        feng.tensor_mul(ft[:], ef, xt)
        f_tiles.append(ft)

    def _numpart(t):
        """T-pass (PE) -> S evac (ACT) -> W -> H (DVE/Pool)."""
        ftp = ps_b.tile([C, 1024], F32, tag="b", name=f"ftp{t}")
        for half in range(2):
            lo = half * 512
            n = 512 if half == 0 else 388
            for dt in range(3):
                nc.tensor.matmul(ftp[:, lo:lo + n], idm,
                                 f_tiles[t + dt][:, lo:lo + n],
                                 start=(dt == 0), stop=(dt == 2))
        st = sb_s.tile([C, NF], F16, tag="s", name=f"st{t}")
        nc.scalar.activation(st[:], ftp[:, 0:900], AF.Copy)

        sv = st[:].rearrange("p (y x) -> p y x", y=HP)
        weng = nc.gpsimd if t in w_pool else nc.vector
        w1 = sb_tmp.tile([C, HP, WO], F16, tag="w1", name=f"w1_{t}")
        weng.tensor_add(w1[:], sv[:, :, 0:28], sv[:, :, 1:29])
        w2 = sb_tmp.tile([C, HP, WO], F16, tag="w2", name=f"w2_{t}")
        weng.tensor_add(w2[:], w1[:], sv[:, :, 2:30])

        heng = nc.gpsimd if t in h_pool else nc.vector
        h1 = sb_tmp.tile([C, HO, WO], F16, tag="h1", name=f"h1_{t}")
        heng.tensor_add(h1[:], w2[:, 0:28, :], w2[:, 1:29, :])
        numt = sb_tmp.tile([C, HO, WO], F16, tag="numt", name=f"numt{t}")
        heng.tensor_add(numt[:], h1[:], w2[:, 2:30, :])
        num_tiles[t] = numt

    _load(0)
    nc.sync.dma_start(out=cst_t[:], in_=d_cst[:])
    nc.gpsimd.tensor_scalar_add(bias[0:C, 0:1], cst_t[:, 392:393], 0.0)
    for f in range(1, TIN):
        _load(f)
    nc.sync.dma_start(out=selr_t[:], in_=d_selr[:])

    # creation order fixes the PSUM rotation; priority gaps let the tail
    # ops (issued later) slot in right behind their frame's numpart
    prio_gap = {}
    for f in range(TIN):
        _frame(f)
    for t in range(TOUT):
        _numpart(t)
        if t == 0:
            prio_den = tc.cur_priority
            tc.cur_priority += 24
        prio_gap[t] = tc.cur_priority
        tc.cur_priority += 24

    # ---- denominator: 3 dy-packs (rows 12dy+6h+t), 3 dx matmuls, one
    # fused reciprocal ----  (priority: right after tail0's slot)
    saved_p = tc.cur_priority
    tc.cur_priority = prio_den
    ebase = e128[:]
    epbase = ep36[:]
    estride = ebase.ap[0][0]
    epstride = epbase.ap[0][0]
    for dy in range(3):
        src = bass.AP(tensor=ebase.tensor, offset=ebase.offset + 30 * dy,
                      ap=[[estride, 2], [NF, TIN], [1, 840]])
        dst = bass.AP(tensor=epbase.tensor,
                      offset=epbase.offset + 12 * dy * epstride,
                      ap=[[epstride, 12], [1, 840]])
        nc.sync.dma_start(out=dst, in_=src)

    d8p = ps_a.tile([8, 1024], F32, tag="a", name="d8p")
    for ch in range(2):
        ylo = 14 * ch
        for dx in range(3):
            rhs = bass.AP(tensor=epbase.tensor,
                          offset=epbase.offset + 30 * ylo + dx,
                          ap=[[epstride, 36], [30, 14], [1, WO]])
            nc.tensor.matmul(d8p[:, 512 * ch:512 * ch + 392], sel36, rhs,
                             start=(dx == 0), stop=(dx == 2))
    r8f = sb_ep.tile([8, NOF], F32, tag="r8f")
    d8v = d8p[:].rearrange("p (b k) -> p b k", b=2)[:, :, 0:392]
    r8v = r8f[:].rearrange("p (b k) -> p b k", b=2)
    nc.vector.reciprocal_approx_fast(r8v, d8v)
    r8t = sb_ep.tile([8, NOF], F32R, tag="r8")
    nc.vector.tensor_scalar_add(r8t[:], r8f[:], 0.0)
    r8 = r8t[:]
    tc.cur_priority = saved_p

    def _tail(t):
        nv = num_tiles[t][:].rearrange("p y x -> p (y x)")
        mt = sb_m.tile([C, NOF], F16, tag="m", name=f"mt{t}")
        yt = sb_y.tile([C, NOF], F16, tag="y", name=f"yt{t}")

        rp = ps_a.tile([C, 1024], F32, tag="a", name=f"rp{t}")
        for ch in range(2):
            nc.tensor.matmul(rp[:, 512 * ch:512 * ch + 392], selr[:, t, :],
                             r8[:, ch * 392:ch * 392 + 392],
                             start=True, stop=True)
        rpv = rp[:].rearrange("p (b k) -> p b k", b=2)[:, :, 0:392]
        nc.vector.tensor_mul(mt[:].rearrange("p (b k) -> p b k", b=2), nv, rpv)

        yp = ps_a.tile([C, 1024], F32, tag="a", name=f"yp{t}")
        for ch in range(2):
            nc.tensor.matmul(yp[:, 512 * ch:512 * ch + 392], wt,
                             mt[:, ch * 392:ch * 392 + 392],
                             start=True, stop=True)
        ypv = yp[:].rearrange("p (b k) -> p b k", b=2)[:, :, 0:392]
        nc.scalar.activation(yt[:].rearrange("p (b k) -> p b k", b=2), ypv,
                             AF.Identity, bias=bias, scale=1.0)
        nc.scalar.dma_start(out=d_ys[t][:], in_=yt[:])

    # tails: creation order after everything (PSUM rotation), priority
    # slotted right behind each frame's numpart
    for t in range(TOUT):
        saved_p = tc.cur_priority
        tc.cur_priority = prio_gap[t]
        _tail(t)
        tc.cur_priority = saved_p


# ---------------------------------------------------------------------------
# Host side
# ---------------------------------------------------------------------------

def _host_prep(x, q, W_out, b_out):
    B, C_, T, H, W = x.shape
    heads, hs = 2, 64
    xpad = np.zeros((B, C_, T + 2, HP, WP), np.float16)
    xpad[:, :, 1:T + 1, 1:H + 1, 1:W + 1] = np.asarray(x, np.float32)

    cidx = np.arange(C_)
    qfull = (np.asarray(q, np.float32)[cidx % heads, 0, cidx // heads] / hs)
    qm = np.zeros((C_, C_), np.float32)
    for m in range(C_):
        qm[:, m] = np.where(cidx % heads == m % heads, qfull, 0.0)
    cst = np.zeros((C_, 393), np.float16)
    cst[:, 0:128] = qm
    cst[:, 128:256] = np.eye(C_)
    cst[:, 256:384] = np.asarray(W_out, np.float32).T
    cst[:, 392] = np.asarray(b_out, np.float32)

    # sel36[12dy + 6h + t, 4h + tp] = 1 iff 0 <= t - tp <= 2
    for dy in range(3):
        for h in range(2):
            for t in range(TIN):
                for tp in range(TOUT):
                    if 0 <= t - tp <= 2:
                        cst[12 * dy + 6 * h + t, 384 + 4 * h + tp] = 1.0
    # selr[4h + tp, tp, c] = 1 iff c % heads == h   (r-broadcast select)
    selr = np.zeros((8, TOUT, C_), np.float32)
    for tp in range(TOUT):
        selr[4 * (cidx % heads) + tp, tp, cidx] = 1.0

    shared = {"cst": cst, "selr": selr.reshape(8, TOUT * C_)}
    in_maps = []
    for core in range(8):
        b, t0 = core // 4, (core % 4) * 4
        xp = np.ascontiguousarray(
            xpad[b, :, t0:t0 + TIN].reshape(C_, TIN, NF))
        in_maps.append({"xp": xp, **shared})
    return in_maps


_NC_CACHE = {}


def _get_nc(reps=1):
    if reps not in _NC_CACHE:
        _NC_CACHE[reps] = _build_nc(reps=reps)
    return _NC_CACHE[reps]


def kernel(x, q, W_out, b_out):
    x = np.asarray(x, np.float32)
    in_maps = _host_prep(x, q, W_out, b_out)
    nc = _get_nc()
    res = bass_utils.run_bass_kernel_spmd(nc, in_maps, list(range(8)))
    y = np.zeros((2, 128, 16, 28, 28), np.float32)
    for core in range(8):
        b, t0 = core // 4, (core % 4) * 4
        for t in range(TOUT):
            y[b, :, t0 + t] = np.asarray(
                res.results[core][f"y{t}"], np.float32).reshape(C, HO, WO)
    return y


# revision 42
# speedup vs baseline: 1.8979x; 1.0161x over previous
"""AttentiveConv3d (sparse_attention) Trainium2 kernel — self-contained.

kernel(**inputs) takes the FULL inputs
    x     [2, 128, 16, 28, 28] f32
    q     [2, 1, 64] f32
    W_out [128, 128] f32
    b_out [128] f32
and returns the FULL output [2, 128, 16, 28, 28] f32.

Sharding: data-parallel over (batch, T-chunks): 8 cores, core i handles
batch i//4, output frames 4*(i%4) .. 4*(i%4)+3, with a 1-frame halo supplied
by host-side padding/slicing (no device collectives needed).

Math (equivalent to the reference; softmax computed without max-subtraction,
valid because |logits| < ~0.2 for this operator's scaling):
    z   = qmask^T @ xp        (per padded location; both heads)
    E   = exp(z);  F = E * xp
    num = Box3x3x3(F); d = Box3x3x3(E)    (separable box filters)
    y   = W_out @ (num / d) + b_out

v4: fp16 fields (f32 PSUM); host-padded input so exp(z)=1 / F=0 pads are
free; 36-row denominator pack (dy baked into 3 SBUF-SBUF DMAs, dx via 3
accumulating matmuls, one fused reciprocal); loads split across the SP and
ACT DMA queues to beat the ~650ns/DMA issue rate; PSUM pair tiles so the
merge/evacuate stages run as one op per frame; Pool (gpsimd) takes the
off-critical-path F multiplies and the f32r cast.
"""
from contextlib import ExitStack

import numpy as np

import concourse.bass as bass
import concourse.tile as tile
from concourse import bacc, mybir
from concourse import bass_utils

F32 = mybir.dt.float32
F32R = mybir.dt.float32r
F16 = mybir.dt.float16
AF = mybir.ActivationFunctionType

C = 128
TIN, TOUT = 6, 4
HP, WP = 30, 30
HO, WO = 28, 28
NF = HP * WP        # 900
NOF = HO * WO       # 784


def _build_nc(num_devices=8, reps=1, n_warm=4,
              f_pool=(0, 1), w_pool=(), h_pool=(), tw_pe=(2, 3)):
    nc = bacc.Bacc("TRN2", target_bir_lowering=False, debug=False,
                   num_devices=num_devices)
    d_xp = nc.dram_tensor("xp", [C, TIN, NF], F16, kind="ExternalInput").ap()
    # cst: qm | idm | wt | sel36 (rows 0:36 of col 384:392) | bias f16 (col 392)
    d_cst = nc.dram_tensor("cst", [C, 393], F16, kind="ExternalInput").ap()
    d_selr = nc.dram_tensor("selr", [8, TOUT * C], F32R, kind="ExternalInput").ap()
    d_ys = [nc.dram_tensor(f"y{t}", [C, NOF], F16, kind="ExternalOutput").ap()
            for t in range(TOUT)]

    with tile.TileContext(nc) as tc:
        with ExitStack() as ctx:
            consts = ctx.enter_context(tc.tile_pool(name="consts", bufs=1))
            sb_x = ctx.enter_context(tc.tile_pool(name="sb_x", bufs=1))
            sb_e = ctx.enter_context(tc.tile_pool(name="sb_e", bufs=1))
            sb_f = ctx.enter_context(tc.tile_pool(name="sb_f", bufs=1))
            sb_s = ctx.enter_context(tc.tile_pool(name="sb_s", bufs=3))
            sb_tmp = ctx.enter_context(tc.tile_pool(name="sb_tmp", bufs=3))
            sb_m = ctx.enter_context(tc.tile_pool(name="sb_m", bufs=2))
            sb_y = ctx.enter_context(tc.tile_pool(name="sb_y", bufs=4))
            sb_ep = ctx.enter_context(tc.tile_pool(name="sb_ep", bufs=1))
            ps_a = ctx.enter_context(tc.tile_pool(name="ps_a", bufs=2, space="PSUM"))
            ps_b = ctx.enter_context(tc.tile_pool(name="ps_b", bufs=2, space="PSUM"))

            cst_t = consts.tile([C, 393], F16)
            selr_t = consts.tile([8, TOUT * C], F32R)
            bias_t = consts.tile([C, 1], F32)

            # Warm-up: ramp the PE p-state on a memset dummy (no DMA dep).
            wrm_sb = consts.tile([C, 512], F16)
            nc.gpsimd.memset(wrm_sb[:], 0.0)
            for i in range(n_warm):
                wrm = ps_a.tile([C, 1024], F32, tag="a", name=f"warm{i}")
                nc.tensor.matmul(wrm[:, 0:512], wrm_sb[:, 0:128], wrm_sb[:],
                                 start=True, stop=True)

            qm = cst_t[:, 0:128]
            idm = cst_t[:, 128:256]
            wt = cst_t[:, 256:384]
            sel36 = cst_t[0:36, 384:392]
            bias = bias_t[:]
            selr = selr_t[:].rearrange("p (t c) -> p t c", t=TOUT)

            for _ in range(reps):
                _body(tc, nc, d_xp, d_ys, d_selr, d_cst,
                      qm, idm, wt, bias, selr, selr_t, sel36, cst_t,
                      sb_x, sb_e, sb_f, sb_s, sb_tmp, sb_m, sb_y, sb_ep,
                      ps_a, ps_b, f_pool, w_pool, h_pool, tw_pe)
    nc.compile()
    return nc


def _body(tc, nc, d_xp, d_ys, d_selr, d_cst,
          qm, idm, wt, bias, selr, selr_t, sel36, cst_t,
          sb_x, sb_e, sb_f, sb_s, sb_tmp, sb_m, sb_y, sb_ep,
          ps_a, ps_b, f_pool, w_pool, h_pool, tw_pe):
    e128 = sb_e.tile([C, TIN * NF], F16, tag="e128")
    ep36 = sb_ep.tile([36, NF], F16, tag="ep36")
    f_tiles = []
    num_tiles = {}
    x_tiles = {}

    def _load(f):
        xtt = sb_x.tile([C, NF], F16, tag=f"x{f}", name=f"xt{f}")
        nc.sync.dma_start(out=xtt[:], in_=d_xp[:, f])
        x_tiles[f] = xtt[:]

    def _frame(f):
        """z matmul, E = exp(z), F = E*xp."""
        xt = x_tiles[f]
        zp = ps_a.tile([C, 1024], F32, tag="a", name=f"zp{f}")
        nc.tensor.matmul(zp[:, 0:512], qm, xt[:, 0:512], start=True, stop=True)
        nc.tensor.matmul(zp[:, 512:900], qm, xt[:, 512:900], start=True, stop=True)

        # exp over the full padded frame: z=0 at pads -> E=1 there for free
        ef = e128[:, f * NF:(f + 1) * NF]
        nc.scalar.activation(ef, zp[:, 0:900], AF.Exp)

        # F = E * xp over the full frame: xp pads are 0 -> F=0 pads for free
        ft = sb_f.tile([C, NF], F16, tag=f"f{f}", name=f"ft{f}")
        feng = nc.gpsimd if f in f_pool else nc.vector
        feng.tensor_mul(ft[:], ef, xt)
        f_tiles.append(ft)

    def _numpart(t):
        """T-pass (PE) -> S evac (ACT) -> W -> H (DVE/Pool).
        For t in tw_pe the W pass is fused into the PE group (9 taps)."""
        ftp = ps_b.tile([C, 1024], F32, tag="b", name=f"ftp{t}")
        if t in tw_pe:
            for half in range(2):
                lo = half * 512
                k = 0
                for dt in range(3):
                    fap = f_tiles[t + dt][:]
                    for dx in range(3):
                        rhs = bass.AP(tensor=fap.tensor,
                                      offset=fap.offset + 450 * half + dx,
                                      ap=[[fap.ap[0][0], C], [30, 15], [1, 28]])
                        nc.tensor.matmul(ftp[:, lo:lo + 420], idm, rhs,
                                         start=(k == 0), stop=(k == 8))
                        k += 1
            st = sb_s.tile([C, 2, 420], F16, tag="s", name=f"st{t}")
            fv = ftp[:].rearrange("p (b k) -> p b k", b=2)[:, :, 0:420]
            nc.scalar.activation(st[:], fv, AF.Copy)
            w2 = bass.AP(tensor=st.tensor, offset=st[:].offset,
                         ap=[[st[:].ap[0][0], C], [28, HP], [1, WO]])
        else:
            for half in range(2):
                lo = half * 512
                n = 512 if half == 0 else 388
                for dt in range(3):
                    nc.tensor.matmul(ftp[:, lo:lo + n], idm,
                                     f_tiles[t + dt][:, lo:lo + n],
                                     start=(dt == 0), stop=(dt == 2))
            st = sb_s.tile([C, NF], F16, tag="s", name=f"st{t}")
            nc.scalar.activation(st[:], ftp[:, 0:900], AF.Copy)

            sv = st[:].rearrange("p (y x) -> p y x", y=HP)
            weng = nc.gpsimd if t in w_pool else nc.vector
            w1 = sb_tmp.tile([C, HP, WO], F16, tag="w1", name=f"w1_{t}")
            weng.tensor_add(w1[:], sv[:, :, 0:28], sv[:, :, 1:29])
            w2t = sb_tmp.tile([C, HP, WO], F16, tag="w2", name=f"w2_{t}")
            weng.tensor_add(w2t[:], w1[:], sv[:, :, 2:30])
            w2 = w2t[:]

        heng = nc.gpsimd if t in h_pool else nc.vector
        h1 = sb_tmp.tile([C, HO, WO], F16, tag="h1", name=f"h1_{t}")
        heng.tensor_add(h1[:], w2[0:C, 0:28, :], w2[0:C, 1:29, :])
        numt = sb_tmp.tile([C, HO, WO], F16, tag="numt", name=f"numt{t}")
        heng.tensor_add(numt[:], h1[:], w2[0:C, 2:30, :])
        num_tiles[t] = numt

    _load(0)
    nc.sync.dma_start(out=cst_t[:], in_=d_cst[:])
    nc.gpsimd.tensor_scalar_add(bias[0:C, 0:1], cst_t[:, 392:393], 0.0)
    for f in range(1, TIN):
        _load(f)
    nc.sync.dma_start(out=selr_t[:], in_=d_selr[:])

    # creation order fixes the PSUM rotation; priority gaps let the tail
    # ops (issued later) slot in right behind their frame's numpart
    prio_gap = {}
    for f in range(TIN):
        _frame(f)
    for t in range(TOUT):
        _numpart(t)
        if t == 0:
            prio_den = tc.cur_priority
            tc.cur_priority += 24
        prio_gap[t] = tc.cur_priority
        tc.cur_priority += 24

    # ---- denominator: 3 dy-packs (rows 12dy+6h+t), 3 dx matmuls, one
    # fused reciprocal ----  (priority: right after tail0's slot)
    saved_p = tc.cur_priority
    tc.cur_priority = prio_den
    ebase = e128[:]
    epbase = ep36[:]
    estride = ebase.ap[0][0]
    epstride = epbase.ap[0][0]
    for dy in range(3):
        src = bass.AP(tensor=ebase.tensor, offset=ebase.offset + 30 * dy,
                      ap=[[estride, 2], [NF, TIN], [1, 840]])
        dst = bass.AP(tensor=epbase.tensor,
                      offset=epbase.offset + 12 * dy * epstride,
                      ap=[[epstride, 12], [1, 840]])
        nc.sync.dma_start(out=dst, in_=src)

    d8p = ps_a.tile([8, 1024], F32, tag="a", name="d8p")
    for ch in range(2):
        ylo = 14 * ch
        for dx in range(3):
            rhs = bass.AP(tensor=epbase.tensor,
                          offset=epbase.offset + 30 * ylo + dx,
                          ap=[[epstride, 36], [30, 14], [1, WO]])
            nc.tensor.matmul(d8p[:, 512 * ch:512 * ch + 392], sel36, rhs,
                             start=(dx == 0), stop=(dx == 2))
    r8f = sb_ep.tile([8, NOF], F32, tag="r8f")
    d8v = d8p[:].rearrange("p (b k) -> p b k", b=2)[:, :, 0:392]
    r8v = r8f[:].rearrange("p (b k) -> p b k", b=2)
    nc.vector.reciprocal_approx_fast(r8v, d8v)
    r8t = sb_ep.tile([8, NOF], F32R, tag="r8")
    nc.vector.tensor_scalar_add(r8t[:], r8f[:], 0.0)
    r8 = r8t[:]
    tc.cur_priority = saved_p

    def _tail(t):
        nv = num_tiles[t][:].rearrange("p y x -> p (y x)")
        mt = sb_m.tile([C, NOF], F16, tag="m", name=f"mt{t}")
        yt = sb_y.tile([C, NOF], F16, tag="y", name=f"yt{t}")

        rp = ps_a.tile([C, 1024], F32, tag="a", name=f"rp{t}")
        for ch in range(2):
            nc.tensor.matmul(rp[:, 512 * ch:512 * ch + 392], selr[:, t, :],
                             r8[:, ch * 392:ch * 392 + 392],
                             start=True, stop=True)
        rpv = rp[:].rearrange("p (b k) -> p b k", b=2)[:, :, 0:392]
        nc.vector.tensor_mul(mt[:].rearrange("p (b k) -> p b k", b=2), nv, rpv)

        yp = ps_a.tile([C, 1024], F32, tag="a", name=f"yp{t}")
        for ch in range(2):
            nc.tensor.matmul(yp[:, 512 * ch:512 * ch + 392], wt,
                             mt[:, ch * 392:ch * 392 + 392],
                             start=True, stop=True)
        ypv = yp[:].rearrange("p (b k) -> p b k", b=2)[:, :, 0:392]
        nc.scalar.activation(yt[:].rearrange("p (b k) -> p b k", b=2), ypv,
                             AF.Identity, bias=bias, scale=1.0)
        nc.scalar.dma_start(out=d_ys[t][:], in_=yt[:])

    # tails: creation order after everything (PSUM rotation), priority
    # slotted right behind each frame's numpart
    for t in range(TOUT):
        saved_p = tc.cur_priority
        tc.cur_priority = prio_gap[t]
        _tail(t)
        tc.cur_priority = saved_p


# ---------------------------------------------------------------------------
# Host side
# ---------------------------------------------------------------------------

def _host_prep(x, q, W_out, b_out):
    B, C_, T, H, W = x.shape
    heads, hs = 2, 64
    xpad = np.zeros((B, C_, T + 2, HP, WP), np.float16)
    xpad[:, :, 1:T + 1, 1:H + 1, 1:W + 1] = np.asarray(x, np.float32)

    cidx = np.arange(C_)
    qfull = (np.asarray(q, np.float32)[cidx % heads, 0, cidx // heads] / hs)
    qm = np.zeros((C_, C_), np.float32)
    for m in range(C_):
        qm[:, m] = np.where(cidx % heads == m % heads, qfull, 0.0)
    cst = np.zeros((C_, 393), np.float16)
    cst[:, 0:128] = qm
    cst[:, 128:256] = np.eye(C_)
    cst[:, 256:384] = np.asarray(W_out, np.float32).T
    cst[:, 392] = np.asarray(b_out, np.float32)

    # sel36[12dy + 6h + t, 4h + tp] = 1 iff 0 <= t - tp <= 2
    for dy in range(3):
        for h in range(2):
            for t in range(TIN):
                for tp in range(TOUT):
                    if 0 <= t - tp <= 2:
                        cst[12 * dy + 6 * h + t, 384 + 4 * h + tp] = 1.0
    # selr[4h + tp, tp, c] = 1 iff c % heads == h   (r-broadcast select)
    selr = np.zeros((8, TOUT, C_), np.float32)
    for tp in range(TOUT):
        selr[4 * (cidx % heads) + tp, tp, cidx] = 1.0

    shared = {"cst": cst, "selr": selr.reshape(8, TOUT * C_)}
    in_maps = []
    for core in range(8):
        b, t0 = core // 4, (core % 4) * 4
        xp = np.ascontiguousarray(
            xpad[b, :, t0:t0 + TIN].reshape(C_, TIN, NF))
        in_maps.append({"xp": xp, **shared})
    return in_maps


_NC_CACHE = {}


def _get_nc(reps=1):
    if reps not in _NC_CACHE:
        _NC_CACHE[reps] = _build_nc(reps=reps)
    return _NC_CACHE[reps]


def kernel(x, q, W_out, b_out):
    x = np.asarray(x, np.float32)
    in_maps = _host_prep(x, q, W_out, b_out)
    nc = _get_nc()
    res = bass_utils.run_bass_kernel_spmd(nc, in_maps, list(range(8)))
    y = np.zeros((2, 128, 16, 28, 28), np.float32)
    for core in range(8):
        b, t0 = core // 4, (core % 4) * 4
        for t in range(TOUT):
            y[b, :, t0 + t] = np.asarray(
                res.results[core][f"y{t}"], np.float32).reshape(C, HO, WO)
    return y


# revision 43
# speedup vs baseline: 2.0223x; 1.0656x over previous
"""AttentiveConv3d (sparse_attention) Trainium2 kernel — self-contained.

kernel(**inputs) takes the FULL inputs
    x     [2, 128, 16, 28, 28] f32
    q     [2, 1, 64] f32
    W_out [128, 128] f32
    b_out [128] f32
and returns the FULL output [2, 128, 16, 28, 28] f32.

Sharding: data-parallel over (batch, T-chunks): 8 cores, core i handles
batch i//4, output frames 4*(i%4) .. 4*(i%4)+3, with a 1-frame halo supplied
by host-side padding/slicing (no device collectives needed).

Math (equivalent to the reference; softmax computed without max-subtraction,
valid because |logits| < ~0.2 for this operator's scaling):
    z   = qmask^T @ xp        (per padded location; both heads)
    E   = exp(z);  F = E * xp
    num = Box3x3x3(F); d = Box3x3x3(E)    (separable box filters)
    y   = W_out @ (num / d) + b_out

v4: fp16 fields (f32 PSUM); host-padded input so exp(z)=1 / F=0 pads are
free; 36-row denominator pack (dy baked into 3 SBUF-SBUF DMAs, dx via 3
accumulating matmuls, one fused reciprocal); loads split across the SP and
ACT DMA queues to beat the ~650ns/DMA issue rate; PSUM pair tiles so the
merge/evacuate stages run as one op per frame; Pool (gpsimd) takes the
off-critical-path F multiplies and the f32r cast.
"""
from contextlib import ExitStack

import numpy as np

import concourse.bass as bass
import concourse.tile as tile
from concourse import bacc, mybir
from concourse import bass_utils

F32 = mybir.dt.float32
F32R = mybir.dt.float32r
F16 = mybir.dt.float16
AF = mybir.ActivationFunctionType

C = 128
TIN, TOUT = 6, 4
HP, WP = 30, 30
HO, WO = 28, 28
NF = HP * WP        # 900
NOF = HO * WO       # 784


def _build_nc(num_devices=8, reps=1, n_warm=4,
              f_pool=(0, 1), w_pool=(), h_pool=(), tw_pe=(2, 3)):
    nc = bacc.Bacc("TRN2", target_bir_lowering=False, debug=False,
                   num_devices=num_devices)
    d_xp = nc.dram_tensor("xp", [C, TIN, NF], F16, kind="ExternalInput").ap()
    # cst: qm | idm | wt | sel36 (rows 0:36 of col 384:392) | bias f16 (col 392)
    d_cst = nc.dram_tensor("cst", [C, 393], F16, kind="ExternalInput").ap()
    d_selr = nc.dram_tensor("selr", [8, TOUT * C], F32R, kind="ExternalInput").ap()
    d_ys = [nc.dram_tensor(f"y{t}", [C, NOF], F16, kind="ExternalOutput").ap()
            for t in range(TOUT)]

    with tile.TileContext(nc) as tc:
        with ExitStack() as ctx:
            consts = ctx.enter_context(tc.tile_pool(name="consts", bufs=1))
            sb_x = ctx.enter_context(tc.tile_pool(name="sb_x", bufs=1))
            sb_e = ctx.enter_context(tc.tile_pool(name="sb_e", bufs=1))
            sb_f = ctx.enter_context(tc.tile_pool(name="sb_f", bufs=1))
            sb_s = ctx.enter_context(tc.tile_pool(name="sb_s", bufs=3))
            sb_tmp = ctx.enter_context(tc.tile_pool(name="sb_tmp", bufs=3))
            sb_m = ctx.enter_context(tc.tile_pool(name="sb_m", bufs=2))
            sb_y = ctx.enter_context(tc.tile_pool(name="sb_y", bufs=4))
            sb_ep = ctx.enter_context(tc.tile_pool(name="sb_ep", bufs=1))
            ps_a = ctx.enter_context(tc.tile_pool(name="ps_a", bufs=2, space="PSUM"))
            ps_b = ctx.enter_context(tc.tile_pool(name="ps_b", bufs=2, space="PSUM"))

            cst_t = consts.tile([C, 393], F16)
            selr_t = consts.tile([8, TOUT * C], F32R)
            bias_t = consts.tile([C, 1], F32)

            # Warm-up: ramp the PE p-state on a memset dummy (no DMA dep).
            wrm_sb = consts.tile([C, 512], F16)
            nc.gpsimd.memset(wrm_sb[:], 0.0)
            for i in range(n_warm):
                wrm = ps_a.tile([C, 1024], F32, tag="a", name=f"warm{i}")
                nc.tensor.matmul(wrm[:, 0:512], wrm_sb[:, 0:128], wrm_sb[:],
                                 start=True, stop=True)

            qm = cst_t[:, 0:128]
            idm = cst_t[:, 128:256]
            wt = cst_t[:, 256:384]
            sel36 = cst_t[0:36, 384:392]
            bias = bias_t[:]
            selr = selr_t[:].rearrange("p (t c) -> p t c", t=TOUT)

            for _ in range(reps):
                _body(tc, nc, d_xp, d_ys, d_selr, d_cst,
                      qm, idm, wt, bias, selr, selr_t, sel36, cst_t,
                      sb_x, sb_e, sb_f, sb_s, sb_tmp, sb_m, sb_y, sb_ep,
                      ps_a, ps_b, f_pool, w_pool, h_pool, tw_pe)
    nc.compile()
    return nc


def _body(tc, nc, d_xp, d_ys, d_selr, d_cst,
          qm, idm, wt, bias, selr, selr_t, sel36, cst_t,
          sb_x, sb_e, sb_f, sb_s, sb_tmp, sb_m, sb_y, sb_ep,
          ps_a, ps_b, f_pool, w_pool, h_pool, tw_pe):
    e128 = sb_e.tile([C, TIN * NF], F16, tag="e128")
    ep36 = sb_ep.tile([36, NF], F16, tag="ep36")
    f_tiles = []
    num_tiles = {}
    x_tiles = {}

    def _load(f):
        xtt = sb_x.tile([C, NF], F16, tag=f"x{f}", name=f"xt{f}")
        nc.sync.dma_start(out=xtt[:], in_=d_xp[:, f])
        x_tiles[f] = xtt[:]

    def _frame(f):
        """z matmul, E = exp(z), F = E*xp."""
        xt = x_tiles[f]
        zpool, ztag = (ps_a, "a") if f % 2 == 0 else (ps_b, "b")
        zp = zpool.tile([C, 1024], F32, tag=ztag, name=f"zp{f}")
        nc.tensor.matmul(zp[:, 0:512], qm, xt[:, 0:512], start=True, stop=True)
        nc.tensor.matmul(zp[:, 512:900], qm, xt[:, 512:900], start=True, stop=True)

        # exp over the full padded frame: z=0 at pads -> E=1 there for free
        ef = e128[:, f * NF:(f + 1) * NF]
        nc.scalar.activation(ef, zp[:, 0:900], AF.Exp)

        # F = E * xp over the full frame: xp pads are 0 -> F=0 pads for free
        ft = sb_f.tile([C, NF], F16, tag=f"f{f}", name=f"ft{f}")
        feng = nc.gpsimd if f in f_pool else nc.vector
        feng.tensor_mul(ft[:], ef, xt)
        f_tiles.append(ft)

    def _numpart(t):
        """T-pass (PE) -> S evac (ACT) -> W -> H (DVE/Pool).
        For t in tw_pe the W pass is fused into the PE group (9 taps)."""
        fpool, ftag = (ps_a, "a") if t == 0 else (ps_b, "b")
        ftp = fpool.tile([C, 1024], F32, tag=ftag, name=f"ftp{t}")
        if t in tw_pe:
            for half in range(2):
                lo = half * 512
                k = 0
                for dt in range(3):
                    fap = f_tiles[t + dt][:]
                    for dx in range(3):
                        rhs = bass.AP(tensor=fap.tensor,
                                      offset=fap.offset + 450 * half + dx,
                                      ap=[[fap.ap[0][0], C], [30, 15], [1, 28]])
                        nc.tensor.matmul(ftp[:, lo:lo + 420], idm, rhs,
                                         start=(k == 0), stop=(k == 8))
                        k += 1
            st = sb_s.tile([C, 2, 420], F16, tag="s", name=f"st{t}")
            fv = ftp[:].rearrange("p (b k) -> p b k", b=2)[:, :, 0:420]
            nc.scalar.activation(st[:], fv, AF.Copy)
            w2 = bass.AP(tensor=st.tensor, offset=st[:].offset,
                         ap=[[st[:].ap[0][0], C], [28, HP], [1, WO]])
        else:
            for half in range(2):
                lo = half * 512
                n = 512 if half == 0 else 388
                for dt in range(3):
                    nc.tensor.matmul(ftp[:, lo:lo + n], idm,
                                     f_tiles[t + dt][:, lo:lo + n],
                                     start=(dt == 0), stop=(dt == 2))
            st = sb_s.tile([C, NF], F16, tag="s", name=f"st{t}")
            nc.scalar.activation(st[:], ftp[:, 0:900], AF.Copy)

            sv = st[:].rearrange("p (y x) -> p y x", y=HP)
            weng = nc.gpsimd if t in w_pool else nc.vector
            w1 = sb_tmp.tile([C, HP, WO], F16, tag="w1", name=f"w1_{t}")
            weng.tensor_add(w1[:], sv[:, :, 0:28], sv[:, :, 1:29])
            w2t = sb_tmp.tile([C, HP, WO], F16, tag="w2", name=f"w2_{t}")
            weng.tensor_add(w2t[:], w1[:], sv[:, :, 2:30])
            w2 = w2t[:]

        heng = nc.gpsimd if t in h_pool else nc.vector
        h1 = sb_tmp.tile([C, HO, WO], F16, tag="h1", name=f"h1_{t}")
        heng.tensor_add(h1[:], w2[0:C, 0:28, :], w2[0:C, 1:29, :])
        numt = sb_tmp.tile([C, HO, WO], F16, tag="numt", name=f"numt{t}")
        heng.tensor_add(numt[:], h1[:], w2[0:C, 2:30, :])
        num_tiles[t] = numt

    _load(0)
    nc.sync.dma_start(out=cst_t[:], in_=d_cst[:])
    nc.gpsimd.tensor_scalar_add(bias[0:C, 0:1], cst_t[:, 392:393], 0.0)
    for f in range(1, TIN):
        _load(f)
    nc.sync.dma_start(out=selr_t[:], in_=d_selr[:])

    # creation order fixes the PSUM rotation; priority gaps let the tail
    # ops (issued later) slot in right behind their frame's numpart
    prio_gap = {}
    for f in range(TIN):
        _frame(f)
    for t in range(TOUT):
        _numpart(t)
        if t == 0:
            prio_den = tc.cur_priority
            tc.cur_priority += 24
        prio_gap[t] = tc.cur_priority
        tc.cur_priority += 24

    # ---- denominator: 3 dy-packs (rows 12dy+6h+t), 3 dx matmuls, one
    # fused reciprocal ----  (priority: right after tail0's slot)
    saved_p = tc.cur_priority
    tc.cur_priority = prio_den
    ebase = e128[:]
    epbase = ep36[:]
    estride = ebase.ap[0][0]
    epstride = epbase.ap[0][0]
    for dy in range(3):
        src = bass.AP(tensor=ebase.tensor, offset=ebase.offset + 30 * dy,
                      ap=[[estride, 2], [NF, TIN], [1, 840]])
        dst = bass.AP(tensor=epbase.tensor,
                      offset=epbase.offset + 12 * dy * epstride,
                      ap=[[epstride, 12], [1, 840]])
        nc.sync.dma_start(out=dst, in_=src)

    d8p = ps_a.tile([8, 1024], F32, tag="a", name="d8p")
    for ch in range(2):
        ylo = 14 * ch
        for dx in range(3):
            rhs = bass.AP(tensor=epbase.tensor,
                          offset=epbase.offset + 30 * ylo + dx,
                          ap=[[epstride, 36], [30, 14], [1, WO]])
            nc.tensor.matmul(d8p[:, 512 * ch:512 * ch + 392], sel36, rhs,
                             start=(dx == 0), stop=(dx == 2))
    r8f = sb_ep.tile([8, NOF], F32, tag="r8f")
    d8v = d8p[:].rearrange("p (b k) -> p b k", b=2)[:, :, 0:392]
    r8v = r8f[:].rearrange("p (b k) -> p b k", b=2)
    nc.vector.reciprocal_approx_fast(r8v, d8v)
    r8t = sb_ep.tile([8, NOF], F32R, tag="r8")
    nc.vector.tensor_scalar_add(r8t[:], r8f[:], 0.0)
    r8 = r8t[:]
    tc.cur_priority = saved_p

    def _tail(t):
        nv = num_tiles[t][:].rearrange("p y x -> p (y x)")
        mt = sb_m.tile([C, NOF], F16, tag="m", name=f"mt{t}")
        yt = sb_y.tile([C, NOF], F16, tag="y", name=f"yt{t}")

        tpool, ttag = (ps_a, "a") if t % 2 == 0 else (ps_b, "b")
        rp = tpool.tile([C, 1024], F32, tag=ttag, name=f"rp{t}")
        for ch in range(2):
            nc.tensor.matmul(rp[:, 512 * ch:512 * ch + 392], selr[:, t, :],
                             r8[:, ch * 392:ch * 392 + 392],
                             start=True, stop=True)
        rpv = rp[:].rearrange("p (b k) -> p b k", b=2)[:, :, 0:392]
        nc.vector.tensor_mul(mt[:].rearrange("p (b k) -> p b k", b=2), nv, rpv)

        yp = tpool.tile([C, 1024], F32, tag=ttag, name=f"yp{t}")
        for ch in range(2):
            nc.tensor.matmul(yp[:, 512 * ch:512 * ch + 392], wt,
                             mt[:, ch * 392:ch * 392 + 392],
                             start=True, stop=True)
        ypv = yp[:].rearrange("p (b k) -> p b k", b=2)[:, :, 0:392]
        nc.scalar.activation(yt[:].rearrange("p (b k) -> p b k", b=2), ypv,
                             AF.Identity, bias=bias, scale=1.0)
        nc.scalar.dma_start(out=d_ys[t][:], in_=yt[:])

    # tails: creation order after everything (PSUM rotation), priority
    # slotted right behind each frame's numpart
    for t in range(TOUT):
        saved_p = tc.cur_priority
        tc.cur_priority = prio_gap[t]
        _tail(t)
        tc.cur_priority = saved_p


# ---------------------------------------------------------------------------
# Host side
# ---------------------------------------------------------------------------

def _host_prep(x, q, W_out, b_out):
    B, C_, T, H, W = x.shape
    heads, hs = 2, 64
    xpad = np.zeros((B, C_, T + 2, HP, WP), np.float16)
    xpad[:, :, 1:T + 1, 1:H + 1, 1:W + 1] = np.asarray(x, np.float32)

    cidx = np.arange(C_)
    qfull = (np.asarray(q, np.float32)[cidx % heads, 0, cidx // heads] / hs)
    qm = np.zeros((C_, C_), np.float32)
    for m in range(C_):
        qm[:, m] = np.where(cidx % heads == m % heads, qfull, 0.0)
    cst = np.zeros((C_, 393), np.float16)
    cst[:, 0:128] = qm
    cst[:, 128:256] = np.eye(C_)
    cst[:, 256:384] = np.asarray(W_out, np.float32).T
    cst[:, 392] = np.asarray(b_out, np.float32)

    # sel36[12dy + 6h + t, 4h + tp] = 1 iff 0 <= t - tp <= 2
    for dy in range(3):
        for h in range(2):
            for t in range(TIN):
                for tp in range(TOUT):
                    if 0 <= t - tp <= 2:
                        cst[12 * dy + 6 * h + t, 384 + 4 * h + tp] = 1.0
    # selr[4h + tp, tp, c] = 1 iff c % heads == h   (r-broadcast select)
    selr = np.zeros((8, TOUT, C_), np.float32)
    for tp in range(TOUT):
        selr[4 * (cidx % heads) + tp, tp, cidx] = 1.0

    shared = {"cst": cst, "selr": selr.reshape(8, TOUT * C_)}
    in_maps = []
    for core in range(8):
        b, t0 = core // 4, (core % 4) * 4
        xp = np.ascontiguousarray(
            xpad[b, :, t0:t0 + TIN].reshape(C_, TIN, NF))
        in_maps.append({"xp": xp, **shared})
    return in_maps


_NC_CACHE = {}


def _get_nc(reps=1):
    if reps not in _NC_CACHE:
        _NC_CACHE[reps] = _build_nc(reps=reps)
    return _NC_CACHE[reps]


def kernel(x, q, W_out, b_out):
    x = np.asarray(x, np.float32)
    in_maps = _host_prep(x, q, W_out, b_out)
    nc = _get_nc()
    res = bass_utils.run_bass_kernel_spmd(nc, in_maps, list(range(8)))
    y = np.zeros((2, 128, 16, 28, 28), np.float32)
    for core in range(8):
        b, t0 = core // 4, (core % 4) * 4
        for t in range(TOUT):
            y[b, :, t0 + t] = np.asarray(
                res.results[core][f"y{t}"], np.float32).reshape(C, HO, WO)
    return y
